# revision 1
# baseline (speedup 1.0000x reference)
"""Izhikevich spiking-neuron scan on 8 Trainium2 NeuronCores.

Problem: x[512, 65536] f32 input currents; per step
    v <- (4v^2 + 5v + 1.4 - r + x_t) * DT
    r <- A*(B-1)*DT * v            (memoryless given new v)
    fire = v >= THRESH; v <- C, r <- r + D where fire
output = fire as f32.

Sharding: neurons (axis 1) split 8 ways; each core runs an independent
scan over its 8192-neuron slice — zero communication.

Per-core math (derivation):
  Let u = v/DT. Completing the square removes the linear term:
    u' = 4*DT^2*(u+320)^2 - 25/16 + 1.4 + x - r
  With g = u + 320 and a free scale sigma (G = sigma*g):
    G' = c4*G^2 + w,   c4 = 4*DT^2/sigma
    w_t = sigma*(beta' + x_t) + c_r*G'_{t-1} + (m-term),  c_r = -K*DT
  The recovery kick +D on fire is absorbed into the reset constant
  (reset feeds only the next square, and cross terms vanish since m in {0,1}):
    R*g = sqrt(Rg^2 - D/(4 DT^2)),  Rg = C/DT + 320
  sigma is chosen so threshold - reset == 1 exactly, making the masked
  reset a single fused op:  G_next = min(G', Th) - m.

Per step (5 ops, FD=64):
  q  = G*G                      tensor_tensor mult
  G' = q*c4 + w                 scalar_tensor_tensor
  m  = (G' >= Th)               tensor_scalar is_ge   -> output slab
  z  = G'*c_r + PRE[t+1]        scalar_tensor_tensor  (w for next step)
  G  = min(G', Th) - m          scalar_tensor_tensor  (fused fire-reset)
PRE = sigma*x + sigma*beta' is a bulk activation (Copy w/ scale+bias) per chunk.
"""

import math
import os
import sys

import numpy as np

if "/opt/trn_rl_repo" not in sys.path:
    sys.path.insert(0, "/opt/trn_rl_repo")

# ---- problem constants (hardcoded; kernel.py must be self-contained) ----
T = 512
N = 65536
NCORES = 8
NLOC = N // NCORES          # 8192 neurons per core
P = 128                     # SBUF partitions
F = NLOC // P               # 64 free elems per partition
TC = 64                     # timesteps per DMA chunk
NCHUNK = T // TC

A = 0.02
B = 0.2
C = -0.065
D = 0.008
DT = 1.0 / T
THRESH = 0.3

# ---- derived constants (float64 -> float32) ----
K = A * (B - 1.0) * DT
_beta0 = 320.0 - 25.0 / 16.0 + 1.4
_Thg = THRESH / DT + 320.0
_Rg = C / DT + 320.0
_Rsg = math.sqrt(_Rg * _Rg - D / (4.0 * DT * DT))
_sigma = 1.0 / (_Thg - _Rsg)
C4 = np.float32(4.0 * DT * DT / _sigma)
C_R = np.float32(-K * DT)
TH_S = np.float32(_sigma * _Thg)
G0 = np.float32(_sigma * _Rg)
PRE_SCALE = np.float32(_sigma)
PRE_BIAS = np.float32(_sigma * (_beta0 + 320.0 * K * DT))

# engine assignment knobs (tuned empirically)
Z_ENGINE = os.environ.get("IZI_Z_ENGINE", "vector")   # O5: 'vector' or 'gpsimd'
PRE_ENGINE = os.environ.get("IZI_PRE_ENGINE", "scalar")  # bulk precompute


def _build_nc(repeats: int = 1):
    import concourse.bacc as bacc
    import concourse.mybir as mybir
    from concourse import tile

    fp32 = mybir.dt.float32
    op = mybir.AluOpType

    nc = bacc.Bacc("TRN2", target_bir_lowering=False)
    x_d = nc.dram_tensor("x", [T, NLOC], fp32, kind="ExternalInput")
    y_d = nc.dram_tensor("spk", [T, NLOC], fp32, kind="ExternalOutput")

    # HBM views: [TC, P*F] rows -> [P, TC, F] (partition-major, 256B runs)
    def chunk_view(dram, ci):
        return dram[ci * TC : (ci + 1) * TC, :].rearrange("t (p f) -> p t f", p=P)

    z_eng_attr = "vector" if Z_ENGINE == "vector" else "gpsimd"

    with tile.TileContext(nc) as tc:
        with (
            tc.tile_pool(name="xin", bufs=2) as xin_pool,
            tc.tile_pool(name="pre", bufs=2) as pre_pool,
            tc.tile_pool(name="out", bufs=2) as out_pool,
            tc.tile_pool(name="state", bufs=2) as g_pool,
            tc.tile_pool(name="gp", bufs=2) as gp_pool,
            tc.tile_pool(name="q", bufs=2) as q_pool,
            tc.tile_pool(name="w", bufs=2) as w_pool,
        ):
            z_eng = getattr(nc, z_eng_attr)
            pre_eng = getattr(nc, PRE_ENGINE)

            pre_tiles = [None] * NCHUNK

            def load_chunk(ci):
                xt = xin_pool.tile([P, TC * F], fp32, tag="xin")
                nc.sync.dma_start(
                    out=xt.rearrange("p (t f) -> p t f", t=TC),
                    in_=chunk_view(x_d, ci),
                )
                pt = pre_pool.tile([P, TC * F], fp32, tag="pre")
                if PRE_ENGINE == "scalar":
                    nc.scalar.activation(
                        pt[:], xt[:],
                        mybir.ActivationFunctionType.Copy,
                        bias=float(PRE_BIAS), scale=float(PRE_SCALE),
                    )
                else:
                    pre_eng.tensor_scalar(
                        pt[:], xt[:], float(PRE_SCALE), float(PRE_BIAS),
                        op.mult, op.add,
                    )
                pre_tiles[ci] = pt

            for _rep in range(repeats):
                # initial state tile
                G = g_pool.tile([P, F], fp32, tag="G")
                nc.vector.memset(G[:], float(G0))
                load_chunk(0)
                w = None  # step-0 w is PRE[0] directly (r_0 = 0)

                for ci in range(NCHUNK):
                    if ci + 1 < NCHUNK:
                        load_chunk(ci + 1)
                    pre = pre_tiles[ci]
                    ot = out_pool.tile([P, TC * F], fp32, tag="out")
                    for tt in range(TC):
                        t = ci * TC + tt
                        win = pre[:, 0:F] if t == 0 else w[:]
                        q = q_pool.tile([P, F], fp32, tag="q")
                        nc.vector.tensor_tensor(q[:], G[:], G[:], op.mult)
                        Gp = gp_pool.tile([P, F], fp32, tag="Gp")
                        nc.vector.scalar_tensor_tensor(
                            Gp[:], q[:], float(C4), win, op.mult, op.add
                        )
                        m = ot[:, tt * F : (tt + 1) * F]
                        nc.vector.tensor_scalar(
                            m, Gp[:], float(TH_S), None, op.is_ge
                        )
                        if t + 1 < T:
                            if tt + 1 < TC:
                                nxt = pre[:, (tt + 1) * F : (tt + 2) * F]
                            else:
                                nxt = pre_tiles[ci + 1][:, 0:F]
                            w = w_pool.tile([P, F], fp32, tag="w")
                            z_eng.scalar_tensor_tensor(
                                w[:], Gp[:], float(C_R), nxt, op.mult, op.add
                            )
                            G = g_pool.tile([P, F], fp32, tag="G")
                            nc.vector.scalar_tensor_tensor(
                                G[:], Gp[:], float(TH_S), m, op.min, op.subtract
                            )
                    # release the x/pre chunk implicitly via pool rotation
                    pre_tiles[ci] = None
                    nc.sync.dma_start(
                        out=chunk_view(y_d, ci),
                        in_=ot.rearrange("p (t f) -> p t f", t=TC),
                    )
    nc.compile()
    return nc


_CACHE: dict = {}


def kernel(x: np.ndarray) -> np.ndarray:
    from concourse.bass_utils import run_bass_kernel_spmd

    x = np.ascontiguousarray(np.asarray(x, np.float32))
    assert x.shape == (T, N), x.shape

    if "nc" not in _CACHE:
        _CACHE["nc"] = _build_nc()
    nc = _CACHE["nc"]

    core_ids = list(range(NCORES))
    in_maps = [
        {"x": np.ascontiguousarray(x[:, c * NLOC : (c + 1) * NLOC])}
        for c in core_ids
    ]
    res = run_bass_kernel_spmd(nc, in_maps, core_ids)
    outs = res.results
    return np.concatenate([outs[c]["spk"] for c in core_ids], axis=1)


if __name__ == "__main__":
    xt = np.random.randn(T, N).astype(np.float32)
    y = kernel(xt)
    print("out", y.shape, y.dtype, y.sum())

